# revision 13
# baseline (speedup 1.0000x reference)
"""Multi-head attention (B=2, S=2048, D=768, H=16, dk=48) on 8 TRN2 NeuronCores.

Sharding: core c = (batch b = c//4, head-group g = c%4 of 4 heads).
Each core computes Q/K/V projections for its 4 heads, full attention over
S=2048, and a partial output projection (contribution of its heads).
Host sums the 4 bf16 partials per batch and adds the analytically-folded
biases (softmax rows sum to 1, so the V-bias contributes Wo @ bv).

Device-side structure (all matmuls bf16, fp32 PSUM accumulation):
- Bias-free device math: bk drops out of softmax entirely (constant per
  query row); bq is folded into an extra K-projection column
  (k'_j = (bq*scale) . k_j, weight w* = Wk_h^T bq_h s at padded col 48)
  paired with a constant-ones row 48 in the Q strips (written by DMA),
  so scores = (q + bq*s) . k_j exactly, with K=49 contraction.
- Exp is the former bottleneck (16.8M elems/core): now split across the
  scalar engine (true Exp) and the vector engine (1-op Schraudolph fast
  exp: bits_i16 = round(x*128/ln2 + 16256-C), bitcast to bf16, ~+-3%
  sawtooth) by a per-tile load-deficit scheduler. PSUM->SBUF evictions
  (q/k/v/ctx/out) are scheduled the same way between ACT and DVE.
- V packed 49 cols/head (48 + denominator ones column), 196/core.
- ctx accumulates transposed in two single-bank PSUM tiles; softmax
  denominator rides along as ctx row 48/112 via the ones columns.
- Normalization: copy ctx to SBUF (frees PSUM), DMA-gather den rows,
  reciprocal_approx_fast (+round to bf16), rank-2 broadcast matmul
  (bf16), fused multiply into bf16 ctxT. The recip+bcast+mul stage and
  the output projection are deferred into later attention steps so the
  in-order engines never idle waiting on the den DMA.
- The attention inner loop is software-pipelined: each step emits
  scores_k, exp_k (half per engine), fill work, ctx_{k-1} - the PE
  never waits a full exp latency (PE idle gaps also drop its clock
  from 2.4 to ~1.2 GHz, so gaps cost double). Measured on HW:
  tile_position strip pairs do NOT execute concurrently (~427ns/pair
  = serial), so the PE floor is ~150us/core and sets the pace.
- PSUM: scores 2x[128,1024] (4 banks) + ctx 2x[128,512] (2) + aux
  2x[128,512] (2) for projections, bcast and out-proj chunks.
"""
import os
import sys
import numpy as np
import ml_dtypes

for _p in ("/opt/trn_rl_repo", "/opt/pypackages"):
    if os.path.isdir(_p) and _p not in sys.path:
        sys.path.append(_p)

import concourse.bacc as bacc
import concourse.mybir as mybir
import concourse.tile as tile
from concourse.bass_utils import run_bass_kernel_spmd

F32 = mybir.dt.float32
F32R = mybir.dt.float32r
BF16 = mybir.dt.bfloat16
I16 = mybir.dt.int16
NPBF16 = ml_dtypes.bfloat16

B = 2
S = 2048
D = 768
H = 16
DK = 48
HPC = 4            # heads per core
NPAIR = 2          # head pairs per core
E = NPAIR * 128    # padded per-core q/k head dim (4 heads x 64)
EV = HPC * 49      # packed v width (48 + ones col per head) = 196
KT = D // 128      # 6 contraction tiles for projections
ST = S // 128      # 16 s-tiles
NQ = 4             # sq quarters
QW = S // NQ       # 512
NCORES = 8

# Schraudolph bf16 fast-exp constants (bits = x*A + B as int16)
EXP_A = 128.0 / float(np.log(2.0))
EXP_C = 5.5        # correction, calibrated end-to-end in simulation
EXP_B = 16256.0 - EXP_C

_PROGRAM = None


class _EngSched:
    """Greedy load-balancer between the scalar (ACT) and vector (DVE)
    engines for PSUM->SBUF work. Costs in ns from the instruction cost
    model (ACT 0.833 ns/col + 217, DVE 1.0417 ns/col + 170)."""

    def __init__(self, nc):
        self.nc = nc
        self.t = {"act": 0.0, "dve": 0.0}

    def _pick(self, ca, cd):
        if self.t["act"] + ca <= self.t["dve"] + cd:
            self.t["act"] += ca
            return "act"
        self.t["dve"] += cd
        return "dve"

    def dve_only(self, cost):
        self.t["dve"] += cost

    def exp(self, ex_ap, sc_ap, eng, free):
        """exp of one half-tile on a FIXED engine (side0->ACT true Exp,
        side1->DVE Schraudolph) so each step's exps finish in ~0.7us on
        two engines in parallel and never gate the PE."""
        nc = self.nc
        with nc.allow_low_precision(reason="probs in bf16"):
            if eng == "act":
                self.t["act"] += 0.8333 * free + 217
                nc.scalar.activation(ex_ap, sc_ap,
                                     mybir.ActivationFunctionType.Exp)
            else:
                self.t["dve"] += 1.0417 * free + 170
                nc.vector.tensor_scalar(
                    out=ex_ap.bitcast(I16), in0=sc_ap,
                    scalar1=float(EXP_A), scalar2=float(EXP_B),
                    op0=mybir.AluOpType.mult, op1=mybir.AluOpType.add)

    def evict(self, dst, src, free):
        eng = self._pick(0.8333 * free + 217, 1.0417 * free + 170)
        nc = self.nc
        with nc.allow_low_precision(reason="bf16 eviction"):
            if eng == "act":
                nc.scalar.copy(dst, src)
            else:
                nc.vector.tensor_copy(dst, src)


def _build_program(variant="full"):
    nc = bacc.Bacc("TRN2", target_bir_lowering=False, debug=False)

    xT = nc.dram_tensor("xT", [D, S], BF16, kind="ExternalInput")
    wq = nc.dram_tensor("wq", [D, E], BF16, kind="ExternalInput")
    wk = nc.dram_tensor("wk", [D, E], BF16, kind="ExternalInput")
    wv = nc.dram_tensor("wv", [D, EV], BF16, kind="ExternalInput")
    wo = nc.dram_tensor("wo", [E, D], BF16, kind="ExternalInput")
    ones2 = nc.dram_tensor("ones2", [2, 128], BF16, kind="ExternalInput")
    out = nc.dram_tensor("out", [S, D], BF16, kind="ExternalOutput")

    with tile.TileContext(nc) as tc:
        with (
            tc.tile_pool(name="xw", bufs=1) as xw,          # x + weights
            tc.tile_pool(name="qkv", bufs=1) as qkv,        # qT/kT/v/ctxT
            tc.tile_pool(name="expp", bufs=8) as expp,      # exp tiles
            tc.tile_pool(name="outp", bufs=4) as outp,      # out staging
            tc.tile_pool(name="misc", bufs=4) as misc,      # ctxu/denom/recip
            tc.tile_pool(name="ps_sc", bufs=2, space="PSUM") as ps_sc,    # 4 banks
            tc.tile_pool(name="ps_ctx", bufs=2, space="PSUM") as ps_ctx,  # 2 banks
            tc.tile_pool(name="ps_aux", bufs=2, space="PSUM") as ps_aux,  # 2 banks
        ):
            sched = _EngSched(nc)

            # ---------- input DMAs ----------
            xT_sb = []
            for k in range(KT):
                t = xw.tile([128, S], BF16, name=f"xT_sb{k}", tag=f"xT_sb{k}")
                # chunked so consumers' deps resolve per 512-column slice
                for c in range(4):
                    nc.sync.dma_start(
                        out=t[:, 512 * c:512 * (c + 1)],
                        in_=xT[128 * k:128 * (k + 1), 512 * c:512 * (c + 1)])
                xT_sb.append(t)

            w_sb = {}
            for nm, dram, w in (("wk", wk, E), ("wq", wq, E), ("wv", wv, EV)):
                tiles = []
                for k in range(KT):
                    t = xw.tile([128, w], BF16, name=f"{nm}_sb{k}",
                                tag=f"{nm}_sb{k}")
                    nc.sync.dma_start(out=t[:], in_=dram[128 * k:128 * (k + 1), :])
                    tiles.append(t)
                w_sb[nm] = tiles

            wo_sb = []
            for k in range(NPAIR):
                t = xw.tile([128, D], BF16, name=f"wo_sb{k}", tag=f"wo_sb{k}")
                nc.sync.dma_start(out=t[:], in_=wo[128 * k:128 * (k + 1), :])
                wo_sb.append(t)

            ones_sb = xw.tile([2, 128], BF16, name="ones_sb", tag="ones_sb")
            nc.sync.dma_start(out=ones_sb[:], in_=ones2[:])

            # bf16 1.0 row, DMA'd into Q-strip rows 48/112 (gpsimd memset;
            # gpsimd is otherwise idle - it cannot touch PSUM)
            onesbf = xw.tile([1, QW], BF16, name="onesbf", tag="onesbf")
            nc.gpsimd.memset(onesbf[:], 1.0)

            # ---------- persistent activations (bf16) ----------
            qT_sb = [qkv.tile([128, S], BF16, name=f"qT_sb{p}", tag=f"qT_sb{p}")
                     for p in range(NPAIR)]
            kT_sb = [qkv.tile([128, S], BF16, name=f"kT_sb{p}", tag=f"kT_sb{p}")
                     for p in range(NPAIR)]
            v_bf = [qkv.tile([128, EV], BF16, name=f"v_bf{st}", tag=f"v_bf{st}")
                    for st in range(ST)]
            ctxT_sb = [qkv.tile([128, S], BF16, name=f"ctxT_sb{p}",
                                tag=f"ctxT_sb{p}")
                       for p in range(NPAIR)]
            # rows 48-63 / 112-127 of ctxT are never written by the
            # normalize muls but are read (x zero weights) by the output
            # projection: zero them so no stale NaN patterns leak through.
            # (gpsimd requires 32-aligned partition ranges, so clear the
            # whole tile; the live rows are overwritten by the muls.)
            for p in range(NPAIR):
                nc.gpsimd.memset(ctxT_sb[p][:], 0.0)

            if variant == "dma":
                junk = outp.tile([128, D], BF16, name="junk", tag="o_sb")
                with nc.allow_low_precision(reason="bench"):
                    nc.vector.tensor_copy(junk[:, 0:S // 4],
                                          xT_sb[0][:, 0:S // 4])
                    for k in range(1, KT):
                        nc.vector.tensor_copy(junk[:, 0:8], xT_sb[k][:, 0:8])
                    for nm2 in ("wk", "wq", "wv"):
                        for k in range(KT):
                            nc.vector.tensor_copy(junk[:, 0:8],
                                                  w_sb[nm2][k][:, 0:8])
                    for k in range(NPAIR):
                        nc.vector.tensor_copy(junk[:, 0:8], wo_sb[k][:, 0:8])
                for st in range(ST):
                    nc.sync.dma_start(out=out[128 * st:128 * (st + 1), :],
                                      in_=junk[:])

            def emit_qk_proj(nm, t, c):
                dst = kT_sb if nm == "wk" else qT_sb
                ps = ps_aux.tile([128, QW], F32, name=f"ps_{nm}{t}_{c}",
                                 tag="ps_aux")
                for k in range(KT):
                    nc.tensor.matmul(
                        ps[:],
                        lhsT=w_sb[nm][k][:, 128 * t:128 * (t + 1)],
                        rhs=xT_sb[k][:, 512 * c:512 * (c + 1)],
                        start=(k == 0), stop=(k == KT - 1),
                    )
                sched.evict(dst[t][:, 512 * c:512 * (c + 1)], ps[:], QW)
                if nm == "wq":
                    # constant-1 rows pairing the k' (folded bq) K-column
                    for r in (48, 112):
                        nc.sync.dma_start(
                            out=dst[t][r:r + 1, 512 * c:512 * (c + 1)],
                            in_=onesbf[:])

            def emit_v_proj(st):
                ps = ps_aux.tile([128, QW], F32, name=f"ps_v{st}", tag="ps_aux")
                psv = ps[:, 0:EV]
                for k in range(KT):
                    nc.tensor.matmul(
                        psv,
                        lhsT=xT_sb[k][:, 128 * st:128 * (st + 1)],
                        rhs=w_sb["wv"][k][:],
                        start=(k == 0), stop=(k == KT - 1),
                    )
                sched.evict(v_bf[st][:], psv, EV)
                for j in range(HPC):
                    nc.gpsimd.memset(v_bf[st][:, 49 * j + 48:49 * j + 49], 1.0)

            if variant == "proj":
                for t in range(NPAIR):
                    for c in range(4):
                        emit_qk_proj("wk", t, c)
                        emit_qk_proj("wq", t, c)
                for st in range(ST):
                    emit_v_proj(st)
                with nc.allow_low_precision(reason="bench"):
                    for st in range(ST):
                        o_sb = outp.tile([128, D], BF16, name=f"o_sb{st}",
                                         tag="o_sb")
                        nc.vector.tensor_copy(o_sb[:, 0:EV], v_bf[st][:])
                        nc.vector.tensor_copy(o_sb[:, 0:D], qT_sb[0][:, 0:D])
                        nc.sync.dma_start(out=out[128 * st:128 * (st + 1), :],
                                          in_=o_sb[:])

            # Prologue: only what step (q0, p0, sk0) needs.
            if variant in ("full", "attn", "noexp"):
                emit_qk_proj("wk", 0, 0)
                emit_qk_proj("wq", 0, 0)

            # Deferred projection chunks: (quarter, pair) -> {sk: [(nm,t,c)]}
            deferred = {
                (0, 0): {2: [("wk", 0, 1)], 5: [("wk", 0, 2)], 8: [("wk", 0, 3)],
                         12: [("wk", 1, 0)], 14: [("wq", 1, 0)]},
                (0, 1): {1: [("wk", 1, 1)], 5: [("wk", 1, 2)],
                         8: [("wk", 1, 3)], 11: [("wq", 0, 1)],
                         13: [("wq", 1, 1)]},
                (1, 1): {2: [("wq", 0, 2)], 8: [("wq", 1, 2)]},
                (2, 1): {2: [("wq", 0, 3)], 8: [("wq", 1, 3)]},
            }

            # ---- deferred work helpers --------------------------------
            def norm_stage2(q, pair, den):
                """reciprocal + broadcast + fused normalize into ctxT"""
                q0 = q * QW
                ctxu = den["ctxu"]
                recf = misc.tile([2, QW], F32, name=f"recf{q}_{pair}",
                                 tag="recf")
                rec = misc.tile([2, QW], BF16, name=f"rec{q}_{pair}", tag="rec")
                nc.vector.reciprocal_approx_fast(out=recf[:], in_=den["den"][:])
                sched.dve_only(1.0417 * QW + 170)
                sched.evict(rec[:], recf[:], QW)
                bc_ps = ps_aux.tile([128, QW], F32, name=f"bc{q}_{pair}",
                                    tag="ps_aux")
                nc.tensor.matmul(bc_ps[:], lhsT=ones_sb[:], rhs=rec[:],
                                 start=True, stop=True)
                with nc.allow_low_precision(reason="bf16 ctxT"):
                    nc.vector.tensor_mul(
                        ctxT_sb[pair][0:48, q0:q0 + QW], ctxu[0:48, :],
                        bc_ps[0:48, :])
                    nc.vector.tensor_mul(
                        ctxT_sb[pair][64:112, q0:q0 + QW], ctxu[64:112, :],
                        bc_ps[64:112, :])
                sched.dve_only(2 * (1.0417 * QW + 170))

            def out_proj_st(st):
                o_sb = outp.tile([128, D], BF16, name=f"o_sb{st}", tag="o_sb")
                for c0, c1 in ((0, 512), (512, D)):
                    ps = ps_aux.tile([128, 512], F32, name=f"ps_o{st}_{c0}",
                                     tag="ps_aux")
                    pso = ps[:, 0:c1 - c0]
                    for k in range(NPAIR):
                        nc.tensor.matmul(
                            pso,
                            lhsT=ctxT_sb[k][:, 128 * st:128 * (st + 1)],
                            rhs=wo_sb[k][:, c0:c1],
                            start=(k == 0), stop=(k == NPAIR - 1),
                        )
                    sched.evict(o_sb[:, c0:c1], pso, c1 - c0)
                nc.sync.dma_start(out=out[128 * st:128 * (st + 1), :],
                                  in_=o_sb[:])

            # (quarter, pair, sk) -> list of deferred thunks, filled as we go
            pending = {}

            def defer(q, pair, sk, fn):
                pending.setdefault((q, pair, sk), []).append(fn)

            def norm_stage1(q, pair, ctx_ps):
                """copy ctx out of PSUM + DMA-gather the denominator rows;
                returns the handles stage 2 needs."""
                ctxu = misc.tile([128, QW], F32, name=f"ctxu{q}_{pair}",
                                 tag="ctxu")
                sched.evict(ctxu[0:49, :], ctx_ps[0:49, :], QW)
                sched.evict(ctxu[64:113, :], ctx_ps[64:113, :], QW)
                den = misc.tile([2, QW], F32, name=f"den{q}_{pair}", tag="den")
                nc.sync.dma_start(out=den[0:1, :], in_=ctxu[48:49, :])
                nc.sync.dma_start(out=den[1:2, :], in_=ctxu[112:113, :])
                return {"ctxu": ctxu, "den": den}

            # ---------- attention, software-pipelined ----------
            # The PE runs in order, so a ctx matmul that waits on its own
            # step's exp would stall it for the whole exp latency. Each
            # step therefore emits: scores_k, exp_k, fill work (deferred
            # projections / normalize stages / out-proj), then ctx_{k-1}
            # whose exp already finished. The ctx pipeline register and
            # the normalize/out-proj defers cross pair/quarter boundaries.
            attn_on = variant in ("full", "attn", "noexp")
            steps = [(q, pair, sk)
                     for q in range(NQ if attn_on else 0)
                     for pair in range(NPAIR) for sk in range(ST)]
            ctx_tiles = {}
            pend_ctx = []

            def emit_ctx(q, pair, sk, ex):
                ctx_ps = ctx_tiles[(q, pair)]
                for side in range(2):
                    nc.tensor.matmul(
                        ctx_ps[64 * side:64 * side + 49, :],
                        lhsT=v_bf[sk][:, 98 * pair + 49 * side:
                                      98 * pair + 49 * side + 49],
                        rhs=ex[:, 512 * side:512 * (side + 1)],
                        start=(sk == 0), stop=(sk == ST - 1),
                        tile_position=(0, 64 * side),
                        skip_group_check=True,
                    )

            for q, pair, sk in steps:
                q0 = q * QW
                if sk == 0:
                    # both head sides share one PSUM bank (the col-strip
                    # matmuls are serial on HW anyway); bufs=2 lets the
                    # next pair start before this pair's stage1 copies.
                    ctx_tiles[(q, pair)] = ps_ctx.tile(
                        [128, QW], F32, name=f"ctx{q}_{pair}", tag="ps_ctx")
                scs = [ps_sc.tile([128, QW], F32,
                                  name=f"sc{q}_{pair}_{sk}_{s}",
                                  tag=f"ps_sc{s}")
                       for s in range(2)]
                for side in range(2):
                    r0 = 64 * side
                    nc.tensor.matmul(
                        scs[side][:],
                        lhsT=kT_sb[pair][r0:r0 + DK + 1,
                                         128 * sk:128 * (sk + 1)],
                        rhs=qT_sb[pair][r0:r0 + DK + 1, q0:q0 + QW],
                        start=True, stop=True,
                        tile_position=(r0, 0),
                    )
                ex = expp.tile([128, 1024], BF16,
                               name=f"ex{q}_{pair}_{sk}", tag="expp")
                if variant == "noexp":
                    nc.vector.memset(ex[0:1, 0:8], 0.5)
                else:
                    sched.exp(ex[:, 0:QW], scs[0][:], "act", QW)
                    sched.exp(ex[:, QW:1024], scs[1][:], "dve", QW)
                pend_ctx.append((q, pair, sk, ex))
                if len(pend_ctx) >= 2:
                    emit_ctx(*pend_ctx.pop(0))
                for nm, t, c in deferred.get((q, pair), {}).get(sk, ()):
                    emit_qk_proj(nm, t, c)
                if q == 0 and pair == 0:
                    emit_v_proj(sk)
                for fn in pending.pop((q, pair, sk), ()):
                    fn()

                if sk == ST - 1:
                    # schedule this pair's epilogue inside the next pair:
                    # stage1 at its step 1 (ctx_15 drains at step 0),
                    # stage2 at step 4 (past the den DMA latency).
                    nq_, np_ = (q, 1) if pair == 0 else (q + 1, 0)
                    in_steps = nq_ < NQ

                    def st1_thunk(qq=q, pp=pair, tq=nq_, tp=np_,
                                  chain=in_steps):
                        hnd = norm_stage1(qq, pp, ctx_tiles.pop((qq, pp)))
                        fn2 = lambda: norm_stage2(qq, pp, hnd)
                        if chain:
                            defer(tq, tp, 5, fn2)
                        else:
                            fn2()

                    if in_steps:
                        defer(nq_, np_, 1, st1_thunk)
                    else:
                        while pend_ctx:            # drain the tail pipeline
                            emit_ctx(*pend_ctx.pop(0))
                        st1_thunk()
                    if pair == 1:
                        for sti in range(QW // 128):
                            st = q * (QW // 128) + sti
                            fn = (lambda s: lambda: out_proj_st(s))(st)
                            if q + 1 < NQ:
                                defer(q + 1, 0, 6 + 3 * sti, fn)
                            else:
                                fn()

    nc.compile()
    return nc


def _build_in_maps(x, Wq, bq, Wk, Wv, Wo):
    scale = 1.0 / np.sqrt(np.float32(DK))
    ones2 = np.zeros((2, 128), NPBF16)
    ones2[0, 0:64] = 1.0
    ones2[1, 64:128] = 1.0

    xT = [np.ascontiguousarray(x[b].T).astype(NPBF16) for b in range(B)]

    in_maps = []
    for core in range(NCORES):
        b, g = divmod(core, HPC)
        wq_pad = np.zeros((D, E), np.float32)
        wk_pad = np.zeros((D, E), np.float32)
        wv_pad = np.zeros((D, EV), np.float32)
        wo_pad = np.zeros((E, D), np.float32)
        for j in range(HPC):
            h = HPC * g + j
            sl = slice(DK * h, DK * (h + 1))
            wq_pad[:, 64 * j:64 * j + DK] = Wq[sl, :].T * scale
            wk_pad[:, 64 * j:64 * j + DK] = Wk[sl, :].T
            wk_pad[:, 64 * j + DK] = Wk[sl, :].T @ (bq[sl] * scale)
            wv_pad[:, 49 * j:49 * j + DK] = Wv[sl, :].T
            wo_pad[64 * j:64 * j + DK, :] = Wo[:, sl].T
        in_maps.append({
            "xT": xT[b],
            "wq": wq_pad.astype(NPBF16),
            "wk": wk_pad.astype(NPBF16),
            "wv": wv_pad.astype(NPBF16),
            "wo": wo_pad.astype(NPBF16),
            "ones2": ones2,
        })
    return in_maps


def _postprocess(results, Wo, bv, bo):
    const = (Wo @ bv + bo).astype(np.float32)  # folded V-bias + out bias
    out = np.empty((B, S, D), np.float32)
    for b in range(B):
        acc = results[HPC * b]["out"].astype(np.float32)
        for g in range(1, HPC):
            acc = acc + results[HPC * b + g]["out"].astype(np.float32)
        out[b] = acc + const
    return out


def get_program():
    global _PROGRAM
    if _PROGRAM is None:
        _PROGRAM = _build_program()
    return _PROGRAM


def kernel(x, Wq, bq, Wk, bk, Wv, bv, Wo, bo):
    x = np.asarray(x, np.float32)
    Wq, bq = np.asarray(Wq, np.float32), np.asarray(bq, np.float32)
    Wk = np.asarray(Wk, np.float32)
    Wv, bv = np.asarray(Wv, np.float32), np.asarray(bv, np.float32)
    Wo, bo = np.asarray(Wo, np.float32), np.asarray(bo, np.float32)

    nc = get_program()
    in_maps = _build_in_maps(x, Wq, bq, Wk, Wv, Wo)
    res = run_bass_kernel_spmd(nc, in_maps, list(range(NCORES)))
    return _postprocess(res.results, Wo, bv, bo)


# revision 15
# speedup vs baseline: 1.0365x; 1.0365x over previous
"""Multi-head attention (B=2, S=2048, D=768, H=16, dk=48) on 8 TRN2 NeuronCores.

Sharding: core c = (batch b = c//4, head-group g = c%4 of 4 heads).
Each core computes Q/K/V projections for its 4 heads, full attention over
S=2048, and a partial output projection (contribution of its heads).
Host sums the 4 bf16 partials per batch and adds the analytically-folded
biases (softmax rows sum to 1, so the V-bias contributes Wo @ bv).

Device-side structure (all matmuls bf16, fp32 PSUM accumulation):
- Bias-free device math: bk drops out of softmax entirely (constant per
  query row); bq is folded into an extra K-projection column
  (k'_j = (bq*scale) . k_j, weight w* = Wk_h^T bq_h s at padded col 48)
  paired with a constant-ones row 48 in the Q strips (written by DMA),
  so scores = (q + bq*s) . k_j exactly, with K=49 contraction.
- Exp is the former bottleneck (16.8M elems/core): now split across the
  scalar engine (true Exp) and the vector engine (1-op Schraudolph fast
  exp: bits_i16 = round(x*128/ln2 + 16256-C), bitcast to bf16, ~+-3%
  sawtooth) by a per-tile load-deficit scheduler. PSUM->SBUF evictions
  (q/k/v/ctx/out) are scheduled the same way between ACT and DVE.
- V packed 49 cols/head (48 + denominator ones column), 196/core.
- ctx accumulates transposed in two single-bank PSUM tiles; softmax
  denominator rides along as ctx row 48/112 via the ones columns.
- Normalization: copy ctx to SBUF (frees PSUM), DMA-gather den rows,
  reciprocal_approx_fast (+round to bf16), rank-2 broadcast matmul
  (bf16), fused multiply into bf16 ctxT. The recip+bcast+mul stage and
  the output projection are deferred into later attention steps so the
  in-order engines never idle waiting on the den DMA.
- The attention inner loop is software-pipelined: each step emits
  scores_k, exp_k (half per engine), fill work, ctx_{k-1} - the PE
  never waits a full exp latency (PE idle gaps also drop its clock
  from 2.4 to ~1.2 GHz, so gaps cost double). Measured on HW:
  tile_position strip pairs do NOT execute concurrently (~427ns/pair
  = serial), so the PE floor is ~150us/core and sets the pace.
- PSUM: scores 4x[128,512] (4 banks, one per exp half) + ctx
  2x[128,512] (2, double-buffered across pairs) + aux 2x[128,512] (2)
  for projections, bcast and out-proj chunks.
"""
import os
import sys
import numpy as np
import ml_dtypes

for _p in ("/opt/trn_rl_repo", "/opt/pypackages"):
    if os.path.isdir(_p) and _p not in sys.path:
        sys.path.append(_p)

import concourse.bacc as bacc
import concourse.mybir as mybir
import concourse.tile as tile
from concourse.bass_utils import run_bass_kernel_spmd

F32 = mybir.dt.float32
F32R = mybir.dt.float32r
BF16 = mybir.dt.bfloat16
I16 = mybir.dt.int16
NPBF16 = ml_dtypes.bfloat16

B = 2
S = 2048
D = 768
H = 16
DK = 48
HPC = 4            # heads per core
NPAIR = 2          # head pairs per core
E = NPAIR * 128    # padded per-core q/k head dim (4 heads x 64)
EV = HPC * 49      # packed v width (48 + ones col per head) = 196
KT = D // 128      # 6 contraction tiles for projections
ST = S // 128      # 16 s-tiles
NQ = 4             # sq quarters
QW = S // NQ       # 512
NCORES = 8

# Schraudolph bf16 fast-exp constants (bits = x*A + B as int16)
EXP_A = 128.0 / float(np.log(2.0))
EXP_C = 5.5        # correction, calibrated end-to-end in simulation
EXP_B = 16256.0 - EXP_C

_PROGRAM = None


class _EngSched:
    """Greedy load-balancer between the scalar (ACT) and vector (DVE)
    engines for PSUM->SBUF work. Costs in ns from the instruction cost
    model (ACT 0.833 ns/col + 217, DVE 1.0417 ns/col + 170)."""

    def __init__(self, nc):
        self.nc = nc
        self.t = {"act": 0.0, "dve": 0.0}

    def _pick(self, ca, cd):
        if self.t["act"] + ca <= self.t["dve"] + cd:
            self.t["act"] += ca
            return "act"
        self.t["dve"] += cd
        return "dve"

    def dve_only(self, cost):
        self.t["dve"] += cost

    def exp(self, ex_ap, sc_ap, eng, free):
        """exp of one half-tile on a FIXED engine (side0->ACT true Exp,
        side1->DVE Schraudolph) so each step's exps finish in ~0.7us on
        two engines in parallel and never gate the PE."""
        nc = self.nc
        with nc.allow_low_precision(reason="probs in bf16"):
            if eng == "act":
                self.t["act"] += 0.8333 * free + 217
                nc.scalar.activation(ex_ap, sc_ap,
                                     mybir.ActivationFunctionType.Exp)
            else:
                self.t["dve"] += 1.0417 * free + 170
                nc.vector.tensor_scalar(
                    out=ex_ap.bitcast(I16), in0=sc_ap,
                    scalar1=float(EXP_A), scalar2=float(EXP_B),
                    op0=mybir.AluOpType.mult, op1=mybir.AluOpType.add)

    def evict(self, dst, src, free):
        eng = self._pick(0.8333 * free + 217, 1.0417 * free + 170)
        nc = self.nc
        with nc.allow_low_precision(reason="bf16 eviction"):
            if eng == "act":
                nc.scalar.copy(dst, src)
            else:
                nc.vector.tensor_copy(dst, src)


def _build_program(variant="full"):
    nc = bacc.Bacc("TRN2", target_bir_lowering=False, debug=False)

    xT = nc.dram_tensor("xT", [D, S], BF16, kind="ExternalInput")
    wq = nc.dram_tensor("wq", [D, E], BF16, kind="ExternalInput")
    wk = nc.dram_tensor("wk", [D, E], BF16, kind="ExternalInput")
    wv = nc.dram_tensor("wv", [D, EV], BF16, kind="ExternalInput")
    wo = nc.dram_tensor("wo", [E, D], BF16, kind="ExternalInput")
    ones2 = nc.dram_tensor("ones2", [2, 128], BF16, kind="ExternalInput")
    out = nc.dram_tensor("out", [S, D], BF16, kind="ExternalOutput")

    with tile.TileContext(nc) as tc:
        with (
            tc.tile_pool(name="xw", bufs=1) as xw,          # x + weights
            tc.tile_pool(name="qkv", bufs=1) as qkv,        # qT/kT/v/ctxT
            tc.tile_pool(name="expp", bufs=8) as expp,      # exp tiles
            tc.tile_pool(name="outp", bufs=4) as outp,      # out staging
            tc.tile_pool(name="misc", bufs=4) as misc,      # ctxu/denom/recip
            tc.tile_pool(name="ps_sc", bufs=2, space="PSUM") as ps_sc,    # 4 banks
            tc.tile_pool(name="ps_ctx", bufs=2, space="PSUM") as ps_ctx,  # 2 banks
            tc.tile_pool(name="ps_aux", bufs=2, space="PSUM") as ps_aux,  # 2 banks
        ):
            sched = _EngSched(nc)

            # ---------- input DMAs ----------
            xT_sb = []
            for k in range(KT):
                t = xw.tile([128, S], BF16, name=f"xT_sb{k}", tag=f"xT_sb{k}")
                # chunked so consumers' deps resolve per 512-column slice
                for c in range(4):
                    nc.sync.dma_start(
                        out=t[:, 512 * c:512 * (c + 1)],
                        in_=xT[128 * k:128 * (k + 1), 512 * c:512 * (c + 1)])
                xT_sb.append(t)

            w_sb = {}
            for nm, dram, w in (("wk", wk, E), ("wq", wq, E), ("wv", wv, EV)):
                tiles = []
                for k in range(KT):
                    t = xw.tile([128, w], BF16, name=f"{nm}_sb{k}",
                                tag=f"{nm}_sb{k}")
                    nc.sync.dma_start(out=t[:], in_=dram[128 * k:128 * (k + 1), :])
                    tiles.append(t)
                w_sb[nm] = tiles

            wo_sb = []
            for k in range(NPAIR):
                t = xw.tile([128, D], BF16, name=f"wo_sb{k}", tag=f"wo_sb{k}")
                nc.sync.dma_start(out=t[:], in_=wo[128 * k:128 * (k + 1), :])
                wo_sb.append(t)

            ones_sb = xw.tile([2, 128], BF16, name="ones_sb", tag="ones_sb")
            nc.sync.dma_start(out=ones_sb[:], in_=ones2[:])

            # bf16 1.0 row, DMA'd into Q-strip rows 48/112 (gpsimd memset;
            # gpsimd is otherwise idle - it cannot touch PSUM)
            onesbf = xw.tile([1, QW], BF16, name="onesbf", tag="onesbf")
            nc.gpsimd.memset(onesbf[:], 1.0)

            # ---------- persistent activations (bf16) ----------
            qT_sb = [qkv.tile([128, S], BF16, name=f"qT_sb{p}", tag=f"qT_sb{p}")
                     for p in range(NPAIR)]
            kT_sb = [qkv.tile([128, S], BF16, name=f"kT_sb{p}", tag=f"kT_sb{p}")
                     for p in range(NPAIR)]
            v_bf = [qkv.tile([128, EV], BF16, name=f"v_bf{st}", tag=f"v_bf{st}")
                    for st in range(ST)]
            ctxT_sb = [qkv.tile([128, S], BF16, name=f"ctxT_sb{p}",
                                tag=f"ctxT_sb{p}")
                       for p in range(NPAIR)]
            # rows 48-63 / 112-127 of ctxT are never written by the
            # normalize muls but are read (x zero weights) by the output
            # projection: zero them so no stale NaN patterns leak through.
            # (gpsimd requires 32-aligned partition ranges, so clear the
            # whole tile; the live rows are overwritten by the muls.)
            for p in range(NPAIR):
                nc.gpsimd.memset(ctxT_sb[p][:], 0.0)

            if variant == "dma":
                junk = outp.tile([128, D], BF16, name="junk", tag="o_sb")
                with nc.allow_low_precision(reason="bench"):
                    nc.vector.tensor_copy(junk[:, 0:S // 4],
                                          xT_sb[0][:, 0:S // 4])
                    for k in range(1, KT):
                        nc.vector.tensor_copy(junk[:, 0:8], xT_sb[k][:, 0:8])
                    for nm2 in ("wk", "wq", "wv"):
                        for k in range(KT):
                            nc.vector.tensor_copy(junk[:, 0:8],
                                                  w_sb[nm2][k][:, 0:8])
                    for k in range(NPAIR):
                        nc.vector.tensor_copy(junk[:, 0:8], wo_sb[k][:, 0:8])
                for st in range(ST):
                    nc.sync.dma_start(out=out[128 * st:128 * (st + 1), :],
                                      in_=junk[:])

            def emit_qk_proj(nm, t, c):
                dst = kT_sb if nm == "wk" else qT_sb
                ps = ps_aux.tile([128, QW], F32, name=f"ps_{nm}{t}_{c}",
                                 tag="ps_aux")
                for k in range(KT):
                    nc.tensor.matmul(
                        ps[:],
                        lhsT=w_sb[nm][k][:, 128 * t:128 * (t + 1)],
                        rhs=xT_sb[k][:, 512 * c:512 * (c + 1)],
                        start=(k == 0), stop=(k == KT - 1),
                    )
                sched.evict(dst[t][:, 512 * c:512 * (c + 1)], ps[:], QW)
                if nm == "wq":
                    # constant-1 rows pairing the k' (folded bq) K-column
                    for r in (48, 112):
                        nc.sync.dma_start(
                            out=dst[t][r:r + 1, 512 * c:512 * (c + 1)],
                            in_=onesbf[:])

            def emit_v_proj(st):
                ps = ps_aux.tile([128, QW], F32, name=f"ps_v{st}", tag="ps_aux")
                psv = ps[:, 0:EV]
                for k in range(KT):
                    nc.tensor.matmul(
                        psv,
                        lhsT=xT_sb[k][:, 128 * st:128 * (st + 1)],
                        rhs=w_sb["wv"][k][:],
                        start=(k == 0), stop=(k == KT - 1),
                    )
                sched.evict(v_bf[st][:], psv, EV)
                for j in range(HPC):
                    nc.gpsimd.memset(v_bf[st][:, 49 * j + 48:49 * j + 49], 1.0)

            if variant == "proj":
                for t in range(NPAIR):
                    for c in range(4):
                        emit_qk_proj("wk", t, c)
                        emit_qk_proj("wq", t, c)
                for st in range(ST):
                    emit_v_proj(st)
                with nc.allow_low_precision(reason="bench"):
                    for st in range(ST):
                        o_sb = outp.tile([128, D], BF16, name=f"o_sb{st}",
                                         tag="o_sb")
                        nc.vector.tensor_copy(o_sb[:, 0:EV], v_bf[st][:])
                        nc.vector.tensor_copy(o_sb[:, 0:D], qT_sb[0][:, 0:D])
                        nc.sync.dma_start(out=out[128 * st:128 * (st + 1), :],
                                          in_=o_sb[:])

            # Prologue: only what step (q0, p0, sk0) needs.
            if variant in ("full", "attn", "noexp"):
                emit_qk_proj("wk", 0, 0)
                emit_qk_proj("wq", 0, 0)

            # Deferred projection chunks: (quarter, pair) -> {sk: [(nm,t,c)]}
            deferred = {
                (0, 0): {2: [("wk", 0, 1)], 5: [("wk", 0, 2)], 8: [("wk", 0, 3)],
                         12: [("wk", 1, 0)], 14: [("wq", 1, 0)]},
                (0, 1): {1: [("wk", 1, 1)], 5: [("wk", 1, 2)],
                         8: [("wk", 1, 3)], 11: [("wq", 0, 1)],
                         13: [("wq", 1, 1)]},
                (1, 1): {2: [("wq", 0, 2)], 8: [("wq", 1, 2)]},
                (2, 1): {2: [("wq", 0, 3)], 8: [("wq", 1, 3)]},
            }

            # ---- deferred work helpers --------------------------------
            def norm_stage2(q, pair, den):
                """reciprocal + broadcast + fused normalize into ctxT"""
                q0 = q * QW
                ctxu = den["ctxu"]
                recf = misc.tile([2, QW], F32, name=f"recf{q}_{pair}",
                                 tag="recf")
                rec = misc.tile([2, QW], BF16, name=f"rec{q}_{pair}", tag="rec")
                nc.vector.reciprocal_approx_fast(out=recf[:], in_=den["den"][:])
                sched.dve_only(1.0417 * QW + 170)
                sched.evict(rec[:], recf[:], QW)
                bc_ps = ps_aux.tile([128, QW], F32, name=f"bc{q}_{pair}",
                                    tag="ps_aux")
                nc.tensor.matmul(bc_ps[:], lhsT=ones_sb[:], rhs=rec[:],
                                 start=True, stop=True)
                with nc.allow_low_precision(reason="bf16 ctxT"):
                    nc.vector.tensor_mul(
                        ctxT_sb[pair][0:48, q0:q0 + QW], ctxu[0:48, :],
                        bc_ps[0:48, :])
                    nc.vector.tensor_mul(
                        ctxT_sb[pair][64:112, q0:q0 + QW], ctxu[64:112, :],
                        bc_ps[64:112, :])
                sched.dve_only(2 * (1.0417 * QW + 170))

            def out_proj_st(st):
                o_sb = outp.tile([128, D], BF16, name=f"o_sb{st}", tag="o_sb")
                for c0, c1 in ((0, 512), (512, D)):
                    ps = ps_aux.tile([128, 512], F32, name=f"ps_o{st}_{c0}",
                                     tag="ps_aux")
                    pso = ps[:, 0:c1 - c0]
                    for k in range(NPAIR):
                        nc.tensor.matmul(
                            pso,
                            lhsT=ctxT_sb[k][:, 128 * st:128 * (st + 1)],
                            rhs=wo_sb[k][:, c0:c1],
                            start=(k == 0), stop=(k == NPAIR - 1),
                        )
                    sched.evict(o_sb[:, c0:c1], pso, c1 - c0)
                nc.sync.dma_start(out=out[128 * st:128 * (st + 1), :],
                                  in_=o_sb[:])

            # (quarter, pair, sk) -> list of deferred thunks, filled as we go
            pending = {}

            def defer(q, pair, sk, fn):
                pending.setdefault((q, pair, sk), []).append(fn)

            def norm_stage1(q, pair, ctx_ps):
                """copy ctx out of PSUM + DMA-gather the denominator rows;
                returns the handles stage 2 needs."""
                ctxu = misc.tile([128, QW], F32, name=f"ctxu{q}_{pair}",
                                 tag="ctxu")
                sched.evict(ctxu[0:49, :], ctx_ps[0:49, :], QW)
                sched.evict(ctxu[64:113, :], ctx_ps[64:113, :], QW)
                den = misc.tile([2, QW], F32, name=f"den{q}_{pair}", tag="den")
                nc.sync.dma_start(out=den[0:1, :], in_=ctxu[48:49, :])
                nc.sync.dma_start(out=den[1:2, :], in_=ctxu[112:113, :])
                return {"ctxu": ctxu, "den": den}

            # ---------- attention, software-pipelined ----------
            # The PE runs in order, so a ctx matmul that waits on its own
            # step's exp would stall it for the whole exp latency. Each
            # step therefore emits: scores_k, exp_k, fill work (deferred
            # projections / normalize stages / out-proj), then ctx_{k-1}
            # whose exp already finished. The ctx pipeline register and
            # the normalize/out-proj defers cross pair/quarter boundaries.
            attn_on = variant in ("full", "attn", "noexp")
            steps = [(q, pair, sk)
                     for q in range(NQ if attn_on else 0)
                     for pair in range(NPAIR) for sk in range(ST)]
            ctx_tiles = {}
            pend_ctx = []

            def emit_ctx(q, pair, sk, ex):
                ctx_ps = ctx_tiles[(q, pair)]
                for side in range(2):
                    nc.tensor.matmul(
                        ctx_ps[64 * side:64 * side + 49, :],
                        lhsT=v_bf[sk][:, 98 * pair + 49 * side:
                                      98 * pair + 49 * side + 49],
                        rhs=ex[:, 512 * side:512 * (side + 1)],
                        start=(sk == 0), stop=(sk == ST - 1),
                        tile_position=(0, 64 * side),
                        skip_group_check=True,
                    )

            for q, pair, sk in steps:
                q0 = q * QW
                if sk == 0:
                    # both head sides share one PSUM bank (the col-strip
                    # matmuls are serial on HW anyway); bufs=2 lets the
                    # next pair start before this pair's stage1 copies.
                    ctx_tiles[(q, pair)] = ps_ctx.tile(
                        [128, QW], F32, name=f"ctx{q}_{pair}", tag="ps_ctx")
                scs = [ps_sc.tile([128, QW], F32,
                                  name=f"sc{q}_{pair}_{sk}_{s}",
                                  tag=f"ps_sc{s}")
                       for s in range(2)]
                for side in range(2):
                    r0 = 64 * side
                    nc.tensor.matmul(
                        scs[side][:],
                        lhsT=kT_sb[pair][r0:r0 + DK + 1,
                                         128 * sk:128 * (sk + 1)],
                        rhs=qT_sb[pair][r0:r0 + DK + 1, q0:q0 + QW],
                        start=True, stop=True,
                        tile_position=(r0, 0),
                    )
                ex = expp.tile([128, 1024], BF16,
                               name=f"ex{q}_{pair}_{sk}", tag="expp")
                if variant == "noexp":
                    nc.vector.memset(ex[0:1, 0:8], 0.5)
                else:
                    sched.exp(ex[:, 0:QW], scs[0][:], "act", QW)
                    sched.exp(ex[:, QW:1024], scs[1][:], "dve", QW)
                for nm, t, c in deferred.get((q, pair), {}).get(sk, ()):
                    emit_qk_proj(nm, t, c)
                if q == 0 and pair == 0:
                    emit_v_proj(sk)
                for fn in pending.pop((q, pair, sk), ()):
                    fn()
                pend_ctx.append((q, pair, sk, ex))
                if len(pend_ctx) >= 2:
                    emit_ctx(*pend_ctx.pop(0))

                if sk == ST - 1:
                    # schedule this pair's epilogue inside the next pair:
                    # stage1 at its step 1 (ctx_15 drains at step 0),
                    # stage2 at step 4 (past the den DMA latency).
                    nq_, np_ = (q, 1) if pair == 0 else (q + 1, 0)
                    in_steps = nq_ < NQ

                    def st1_thunk(qq=q, pp=pair, tq=nq_, tp=np_,
                                  chain=in_steps):
                        hnd = norm_stage1(qq, pp, ctx_tiles.pop((qq, pp)))
                        fn2 = lambda: norm_stage2(qq, pp, hnd)
                        if chain:
                            defer(tq, tp, 4, fn2)
                        else:
                            fn2()

                    if in_steps:
                        defer(nq_, np_, 1, st1_thunk)
                    else:
                        while pend_ctx:            # drain the tail pipeline
                            emit_ctx(*pend_ctx.pop(0))
                        st1_thunk()
                    if pair == 1:
                        for sti in range(QW // 128):
                            st = q * (QW // 128) + sti
                            fn = (lambda s: lambda: out_proj_st(s))(st)
                            if q + 1 < NQ:
                                defer(q + 1, 0, 6 + 3 * sti, fn)
                            else:
                                fn()

    nc.compile()
    return nc


def _build_in_maps(x, Wq, bq, Wk, Wv, Wo):
    scale = 1.0 / np.sqrt(np.float32(DK))
    ones2 = np.zeros((2, 128), NPBF16)
    ones2[0, 0:64] = 1.0
    ones2[1, 64:128] = 1.0

    xT = [np.ascontiguousarray(x[b].T).astype(NPBF16) for b in range(B)]

    in_maps = []
    for core in range(NCORES):
        b, g = divmod(core, HPC)
        wq_pad = np.zeros((D, E), np.float32)
        wk_pad = np.zeros((D, E), np.float32)
        wv_pad = np.zeros((D, EV), np.float32)
        wo_pad = np.zeros((E, D), np.float32)
        for j in range(HPC):
            h = HPC * g + j
            sl = slice(DK * h, DK * (h + 1))
            wq_pad[:, 64 * j:64 * j + DK] = Wq[sl, :].T * scale
            wk_pad[:, 64 * j:64 * j + DK] = Wk[sl, :].T
            wk_pad[:, 64 * j + DK] = Wk[sl, :].T @ (bq[sl] * scale)
            wv_pad[:, 49 * j:49 * j + DK] = Wv[sl, :].T
            wo_pad[64 * j:64 * j + DK, :] = Wo[:, sl].T
        in_maps.append({
            "xT": xT[b],
            "wq": wq_pad.astype(NPBF16),
            "wk": wk_pad.astype(NPBF16),
            "wv": wv_pad.astype(NPBF16),
            "wo": wo_pad.astype(NPBF16),
            "ones2": ones2,
        })
    return in_maps


def _postprocess(results, Wo, bv, bo):
    const = (Wo @ bv + bo).astype(np.float32)  # folded V-bias + out bias
    out = np.empty((B, S, D), np.float32)
    for b in range(B):
        acc = results[HPC * b]["out"].astype(np.float32)
        for g in range(1, HPC):
            acc = acc + results[HPC * b + g]["out"].astype(np.float32)
        out[b] = acc + const
    return out


def get_program():
    global _PROGRAM
    if _PROGRAM is None:
        _PROGRAM = _build_program()
    return _PROGRAM


def kernel(x, Wq, bq, Wk, bk, Wv, bv, Wo, bo):
    x = np.asarray(x, np.float32)
    Wq, bq = np.asarray(Wq, np.float32), np.asarray(bq, np.float32)
    Wk = np.asarray(Wk, np.float32)
    Wv, bv = np.asarray(Wv, np.float32), np.asarray(bv, np.float32)
    Wo, bo = np.asarray(Wo, np.float32), np.asarray(bo, np.float32)

    nc = get_program()
    in_maps = _build_in_maps(x, Wq, bq, Wk, Wv, Wo)
    res = run_bass_kernel_spmd(nc, in_maps, list(range(NCORES)))
    return _postprocess(res.results, Wo, bv, bo)


# revision 16
# speedup vs baseline: 1.0582x; 1.0209x over previous
"""Multi-head attention (B=2, S=2048, D=768, H=16, dk=48) on 8 TRN2 NeuronCores.

Sharding: core c = (batch b = c//4, head-group g = c%4 of 4 heads).
Each core computes Q/K/V projections for its 4 heads, full attention over
S=2048, and a partial output projection (contribution of its heads).
Host sums the 4 bf16 partials per batch and adds the analytically-folded
biases (softmax rows sum to 1, so the V-bias contributes Wo @ bv).

Device-side structure (all matmuls bf16, fp32 PSUM accumulation):
- Bias-free device math: bk drops out of softmax entirely (constant per
  query row); bq is folded into an extra K-projection column
  (k'_j = (bq*scale) . k_j, weight w* = Wk_h^T bq_h s at padded col 48)
  paired with a constant-ones row 48 in the Q strips (written by DMA),
  so scores = (q + bq*s) . k_j exactly, with K=49 contraction.
- Exp is the former bottleneck (16.8M elems/core): now split across the
  scalar engine (true Exp) and the vector engine (1-op Schraudolph fast
  exp: bits_i16 = round(x*128/ln2 + 16256-C), bitcast to bf16, ~+-3%
  sawtooth) by a per-tile load-deficit scheduler. PSUM->SBUF evictions
  (q/k/v/ctx/out) are scheduled the same way between ACT and DVE.
- V packed 49 cols/head (48 + denominator ones column), 196/core.
- ctx accumulates transposed in two single-bank PSUM tiles; softmax
  denominator rides along as ctx row 48/112 via the ones columns.
- Normalization: copy ctx to SBUF (frees PSUM), DMA-gather den rows,
  reciprocal_approx_fast (+round to bf16), rank-2 broadcast matmul
  (bf16), fused multiply into bf16 ctxT. The recip+bcast+mul stage and
  the output projection are deferred into later attention steps so the
  in-order engines never idle waiting on the den DMA.
- The attention inner loop is software-pipelined: each step emits
  scores_k, exp_k (half per engine), fill work, ctx_{k-1} - the PE
  never waits a full exp latency (PE idle gaps also drop its clock
  from 2.4 to ~1.2 GHz, so gaps cost double). Measured on HW:
  tile_position strip pairs do NOT execute concurrently (~427ns/pair
  = serial), so the PE floor is ~150us/core and sets the pace.
- PSUM: scores 4x[128,512] (4 banks, one per exp half) + ctx
  2x[128,512] (2, double-buffered across pairs) + aux 2x[128,512] (2)
  for projections, bcast and out-proj chunks.
"""
import os
import sys
import numpy as np
import ml_dtypes

for _p in ("/opt/trn_rl_repo", "/opt/pypackages"):
    if os.path.isdir(_p) and _p not in sys.path:
        sys.path.append(_p)

import concourse.bacc as bacc
import concourse.mybir as mybir
import concourse.tile as tile
from concourse.bass_utils import run_bass_kernel_spmd

F32 = mybir.dt.float32
F32R = mybir.dt.float32r
BF16 = mybir.dt.bfloat16
I16 = mybir.dt.int16
NPBF16 = ml_dtypes.bfloat16

B = 2
S = 2048
D = 768
H = 16
DK = 48
HPC = 4            # heads per core
NPAIR = 2          # head pairs per core
E = NPAIR * 128    # padded per-core q/k head dim (4 heads x 64)
EV = HPC * 49      # packed v width (48 + ones col per head) = 196
KT = D // 128      # 6 contraction tiles for projections
ST = S // 128      # 16 s-tiles
NQ = 4             # sq quarters
QW = S // NQ       # 512
NCORES = 8

# Schraudolph bf16 fast-exp constants (bits = x*A + B as int16)
EXP_A = 128.0 / float(np.log(2.0))
EXP_C = 5.5        # correction, calibrated end-to-end in simulation
EXP_B = 16256.0 - EXP_C

_PROGRAM = None


class _EngSched:
    """Greedy load-balancer between the scalar (ACT) and vector (DVE)
    engines for PSUM->SBUF work. Costs in ns from the instruction cost
    model (ACT 0.833 ns/col + 217, DVE 1.0417 ns/col + 170)."""

    def __init__(self, nc):
        self.nc = nc
        self.t = {"act": 0.0, "dve": 0.0}

    def _pick(self, ca, cd):
        if self.t["act"] + ca <= self.t["dve"] + cd:
            self.t["act"] += ca
            return "act"
        self.t["dve"] += cd
        return "dve"

    def dve_only(self, cost):
        self.t["dve"] += cost

    def exp(self, ex_ap, sc_ap, eng, free):
        """exp of one half-tile on a FIXED engine (side0->ACT true Exp,
        side1->DVE Schraudolph) so each step's exps finish in ~0.7us on
        two engines in parallel and never gate the PE."""
        nc = self.nc
        with nc.allow_low_precision(reason="probs in bf16"):
            if eng == "act":
                self.t["act"] += 0.8333 * free + 217
                nc.scalar.activation(ex_ap, sc_ap,
                                     mybir.ActivationFunctionType.Exp)
            else:
                self.t["dve"] += 1.0417 * free + 170
                nc.vector.tensor_scalar(
                    out=ex_ap.bitcast(I16), in0=sc_ap,
                    scalar1=float(EXP_A), scalar2=float(EXP_B),
                    op0=mybir.AluOpType.mult, op1=mybir.AluOpType.add)

    def evict(self, dst, src, free):
        eng = self._pick(0.8333 * free + 217, 1.0417 * free + 170)
        nc = self.nc
        with nc.allow_low_precision(reason="bf16 eviction"):
            if eng == "act":
                nc.scalar.copy(dst, src)
            else:
                nc.vector.tensor_copy(dst, src)


def _build_program(variant="full"):
    nc = bacc.Bacc("TRN2", target_bir_lowering=False, debug=False)

    xT = nc.dram_tensor("xT", [D, S], BF16, kind="ExternalInput")
    wq = nc.dram_tensor("wq", [D, E], BF16, kind="ExternalInput")
    wk = nc.dram_tensor("wk", [D, E], BF16, kind="ExternalInput")
    wv = nc.dram_tensor("wv", [D, EV], BF16, kind="ExternalInput")
    wo = nc.dram_tensor("wo", [E, D], BF16, kind="ExternalInput")
    ones2 = nc.dram_tensor("ones2", [2, 128], BF16, kind="ExternalInput")
    out = nc.dram_tensor("out", [S, D], BF16, kind="ExternalOutput")

    with tile.TileContext(nc) as tc:
        with (
            tc.tile_pool(name="xw", bufs=1) as xw,          # x + weights
            tc.tile_pool(name="qkv", bufs=1) as qkv,        # qT/kT/v/ctxT
            tc.tile_pool(name="expp", bufs=8) as expp,      # exp tiles
            tc.tile_pool(name="outp", bufs=4) as outp,      # out staging
            tc.tile_pool(name="misc", bufs=4) as misc,      # ctxu/denom/recip
            tc.tile_pool(name="ps_sc", bufs=2, space="PSUM") as ps_sc,    # 4 banks
            tc.tile_pool(name="ps_ctx", bufs=2, space="PSUM") as ps_ctx,  # 2 banks
            tc.tile_pool(name="ps_aux", bufs=2, space="PSUM") as ps_aux,  # 2 banks
        ):
            sched = _EngSched(nc)

            # ---------- input DMAs ----------
            xT_sb = []
            for k in range(KT):
                t = xw.tile([128, S], BF16, name=f"xT_sb{k}", tag=f"xT_sb{k}")
                # chunked so consumers' deps resolve per 512-column slice
                for c in range(4):
                    nc.sync.dma_start(
                        out=t[:, 512 * c:512 * (c + 1)],
                        in_=xT[128 * k:128 * (k + 1), 512 * c:512 * (c + 1)])
                xT_sb.append(t)

            w_sb = {}
            for nm, dram, w in (("wk", wk, E), ("wq", wq, E), ("wv", wv, EV)):
                tiles = []
                for k in range(KT):
                    t = xw.tile([128, w], BF16, name=f"{nm}_sb{k}",
                                tag=f"{nm}_sb{k}")
                    nc.sync.dma_start(out=t[:], in_=dram[128 * k:128 * (k + 1), :])
                    tiles.append(t)
                w_sb[nm] = tiles

            wo_sb = []
            for k in range(NPAIR):
                t = xw.tile([128, D], BF16, name=f"wo_sb{k}", tag=f"wo_sb{k}")
                nc.sync.dma_start(out=t[:], in_=wo[128 * k:128 * (k + 1), :])
                wo_sb.append(t)

            ones_sb = xw.tile([2, 128], BF16, name="ones_sb", tag="ones_sb")
            nc.sync.dma_start(out=ones_sb[:], in_=ones2[:])

            # bf16 1.0 row, DMA'd into Q-strip rows 48/112 (gpsimd memset;
            # gpsimd is otherwise idle - it cannot touch PSUM)
            onesbf = xw.tile([1, QW], BF16, name="onesbf", tag="onesbf")
            nc.gpsimd.memset(onesbf[:], 1.0)

            # ---------- persistent activations (bf16) ----------
            qT_sb = [qkv.tile([128, S], BF16, name=f"qT_sb{p}", tag=f"qT_sb{p}")
                     for p in range(NPAIR)]
            kT_sb = [qkv.tile([128, S], BF16, name=f"kT_sb{p}", tag=f"kT_sb{p}")
                     for p in range(NPAIR)]
            v_bf = [qkv.tile([128, EV], BF16, name=f"v_bf{st}", tag=f"v_bf{st}")
                    for st in range(ST)]
            ctxT_sb = [qkv.tile([128, S], BF16, name=f"ctxT_sb{p}",
                                tag=f"ctxT_sb{p}")
                       for p in range(NPAIR)]
            # rows 48-63 / 112-127 of ctxT are never written by the
            # normalize muls but are read (x zero weights) by the output
            # projection: zero them so no stale NaN patterns leak through.
            # (gpsimd requires 32-aligned partition ranges, so clear the
            # whole tile; the live rows are overwritten by the muls.)
            for p in range(NPAIR):
                nc.gpsimd.memset(ctxT_sb[p][:], 0.0)

            if variant == "dma":
                junk = outp.tile([128, D], BF16, name="junk", tag="o_sb")
                with nc.allow_low_precision(reason="bench"):
                    nc.vector.tensor_copy(junk[:, 0:S // 4],
                                          xT_sb[0][:, 0:S // 4])
                    for k in range(1, KT):
                        nc.vector.tensor_copy(junk[:, 0:8], xT_sb[k][:, 0:8])
                    for nm2 in ("wk", "wq", "wv"):
                        for k in range(KT):
                            nc.vector.tensor_copy(junk[:, 0:8],
                                                  w_sb[nm2][k][:, 0:8])
                    for k in range(NPAIR):
                        nc.vector.tensor_copy(junk[:, 0:8], wo_sb[k][:, 0:8])
                for st in range(ST):
                    nc.sync.dma_start(out=out[128 * st:128 * (st + 1), :],
                                      in_=junk[:])

            def emit_qk_proj(nm, t, c):
                dst = kT_sb if nm == "wk" else qT_sb
                ps = ps_aux.tile([128, QW], F32, name=f"ps_{nm}{t}_{c}",
                                 tag="ps_aux")
                for k in range(KT):
                    nc.tensor.matmul(
                        ps[:],
                        lhsT=w_sb[nm][k][:, 128 * t:128 * (t + 1)],
                        rhs=xT_sb[k][:, 512 * c:512 * (c + 1)],
                        start=(k == 0), stop=(k == KT - 1),
                    )
                sched.evict(dst[t][:, 512 * c:512 * (c + 1)], ps[:], QW)
                if nm == "wq":
                    # constant-1 rows pairing the k' (folded bq) K-column
                    for r in (48, 112):
                        nc.sync.dma_start(
                            out=dst[t][r:r + 1, 512 * c:512 * (c + 1)],
                            in_=onesbf[:])

            def emit_v_proj(st):
                ps = ps_aux.tile([128, QW], F32, name=f"ps_v{st}", tag="ps_aux")
                psv = ps[:, 0:EV]
                for k in range(KT):
                    nc.tensor.matmul(
                        psv,
                        lhsT=xT_sb[k][:, 128 * st:128 * (st + 1)],
                        rhs=w_sb["wv"][k][:],
                        start=(k == 0), stop=(k == KT - 1),
                    )
                sched.evict(v_bf[st][:], psv, EV)
                for j in range(HPC):
                    nc.gpsimd.memset(v_bf[st][:, 49 * j + 48:49 * j + 49], 1.0)

            if variant == "proj":
                for t in range(NPAIR):
                    for c in range(4):
                        emit_qk_proj("wk", t, c)
                        emit_qk_proj("wq", t, c)
                for st in range(ST):
                    emit_v_proj(st)
                with nc.allow_low_precision(reason="bench"):
                    for st in range(ST):
                        o_sb = outp.tile([128, D], BF16, name=f"o_sb{st}",
                                         tag="o_sb")
                        nc.vector.tensor_copy(o_sb[:, 0:EV], v_bf[st][:])
                        nc.vector.tensor_copy(o_sb[:, 0:D], qT_sb[0][:, 0:D])
                        nc.sync.dma_start(out=out[128 * st:128 * (st + 1), :],
                                          in_=o_sb[:])

            # Prologue: only what step (q0, p0, sk0) needs.
            if variant in ("full", "attn", "noexp"):
                emit_qk_proj("wk", 0, 0)
                emit_qk_proj("wq", 0, 0)

            # Deferred projection chunks: (quarter, pair) -> {sk: [(nm,t,c)]}
            deferred = {
                (0, 0): {2: [("wk", 0, 1)], 5: [("wk", 0, 2)], 8: [("wk", 0, 3)],
                         12: [("wk", 1, 0)], 14: [("wq", 1, 0)]},
                (0, 1): {1: [("wk", 1, 1)], 5: [("wk", 1, 2)],
                         8: [("wk", 1, 3)], 11: [("wq", 0, 1)],
                         13: [("wq", 1, 1)]},
                (1, 1): {2: [("wq", 0, 2)], 8: [("wq", 1, 2)]},
                (2, 1): {2: [("wq", 0, 3)], 8: [("wq", 1, 3)]},
            }

            # ---- deferred work helpers --------------------------------
            def norm_stage2(q, pair, den):
                """reciprocal + broadcast + fused normalize into ctxT"""
                q0 = q * QW
                ctxu = den["ctxu"]
                recf = misc.tile([2, QW], F32, name=f"recf{q}_{pair}",
                                 tag="recf")
                rec = misc.tile([2, QW], BF16, name=f"rec{q}_{pair}", tag="rec")
                nc.vector.reciprocal_approx_fast(out=recf[:], in_=den["den"][:])
                sched.dve_only(1.0417 * QW + 170)
                sched.evict(rec[:], recf[:], QW)
                bc_ps = ps_aux.tile([128, QW], F32, name=f"bc{q}_{pair}",
                                    tag="ps_aux")
                nc.tensor.matmul(bc_ps[:], lhsT=ones_sb[:], rhs=rec[:],
                                 start=True, stop=True)
                with nc.allow_low_precision(reason="bf16 ctxT"):
                    nc.vector.tensor_mul(
                        ctxT_sb[pair][0:48, q0:q0 + QW], ctxu[0:48, :],
                        bc_ps[0:48, :])
                    nc.vector.tensor_mul(
                        ctxT_sb[pair][64:112, q0:q0 + QW], ctxu[64:112, :],
                        bc_ps[64:112, :])
                sched.dve_only(2 * (1.0417 * QW + 170))

            def out_proj_st(st):
                o_sb = outp.tile([128, D], BF16, name=f"o_sb{st}", tag="o_sb")
                for c0, c1 in ((0, 512), (512, D)):
                    ps = ps_aux.tile([128, 512], F32, name=f"ps_o{st}_{c0}",
                                     tag="ps_aux")
                    pso = ps[:, 0:c1 - c0]
                    for k in range(NPAIR):
                        nc.tensor.matmul(
                            pso,
                            lhsT=ctxT_sb[k][:, 128 * st:128 * (st + 1)],
                            rhs=wo_sb[k][:, c0:c1],
                            start=(k == 0), stop=(k == NPAIR - 1),
                        )
                    sched.evict(o_sb[:, c0:c1], pso, c1 - c0)
                nc.sync.dma_start(out=out[128 * st:128 * (st + 1), :],
                                  in_=o_sb[:])

            # (quarter, pair, sk) -> list of deferred thunks, filled as we go
            pending = {}

            def defer(q, pair, sk, fn):
                pending.setdefault((q, pair, sk), []).append(fn)

            def norm_stage1(q, pair, ctx_ps):
                """copy ctx out of PSUM + DMA-gather the denominator rows;
                returns the handles stage 2 needs."""
                ctxu = misc.tile([128, QW], F32, name=f"ctxu{q}_{pair}",
                                 tag="ctxu")
                sched.evict(ctxu[0:49, :], ctx_ps[0:49, :], QW)
                sched.evict(ctxu[64:113, :], ctx_ps[64:113, :], QW)
                den = misc.tile([2, QW], F32, name=f"den{q}_{pair}", tag="den")
                nc.sync.dma_start(out=den[0:1, :], in_=ctxu[48:49, :])
                nc.sync.dma_start(out=den[1:2, :], in_=ctxu[112:113, :])
                return {"ctxu": ctxu, "den": den}

            # ---------- attention, software-pipelined ----------
            # The PE runs in order, so a ctx matmul that waits on its own
            # step's exp would stall it for the whole exp latency. Each
            # step therefore emits: scores_k, exp_k, fill work (deferred
            # projections / normalize stages / out-proj), then ctx_{k-1}
            # whose exp already finished. The ctx pipeline register and
            # the normalize/out-proj defers cross pair/quarter boundaries.
            attn_on = variant in ("full", "attn", "noexp")
            steps = [(q, pair, sk)
                     for q in range(NQ if attn_on else 0)
                     for pair in range(NPAIR) for sk in range(ST)]
            ctx_tiles = {}
            pend_ctx = []

            def emit_ctx(q, pair, sk, ex):
                ctx_ps = ctx_tiles[(q, pair)]
                for side in range(2):
                    nc.tensor.matmul(
                        ctx_ps[64 * side:64 * side + 49, :],
                        lhsT=v_bf[sk][:, 98 * pair + 49 * side:
                                      98 * pair + 49 * side + 49],
                        rhs=ex[:, 512 * side:512 * (side + 1)],
                        start=(sk == 0), stop=(sk == ST - 1),
                        tile_position=(0, 64 * side),
                        skip_group_check=True,
                    )

            for q, pair, sk in steps:
                q0 = q * QW
                if sk == 0:
                    # both head sides share one PSUM bank (the col-strip
                    # matmuls are serial on HW anyway); bufs=2 lets the
                    # next pair start before this pair's stage1 copies.
                    ctx_tiles[(q, pair)] = ps_ctx.tile(
                        [128, QW], F32, name=f"ctx{q}_{pair}", tag="ps_ctx")
                scs = [ps_sc.tile([128, QW], F32,
                                  name=f"sc{q}_{pair}_{sk}_{s}",
                                  tag=f"ps_sc{s}")
                       for s in range(2)]
                for side in range(2):
                    r0 = 64 * side
                    nc.tensor.matmul(
                        scs[side][:],
                        lhsT=kT_sb[pair][r0:r0 + DK + 1,
                                         128 * sk:128 * (sk + 1)],
                        rhs=qT_sb[pair][r0:r0 + DK + 1, q0:q0 + QW],
                        start=True, stop=True,
                        tile_position=(r0, 0),
                    )
                ex = expp.tile([128, 1024], BF16,
                               name=f"ex{q}_{pair}_{sk}", tag="expp")
                if variant == "noexp":
                    nc.vector.memset(ex[0:1, 0:8], 0.5)
                else:
                    sched.exp(ex[:, 0:QW], scs[0][:], "act", QW)
                    sched.exp(ex[:, QW:1024], scs[1][:], "dve", QW)
                for nm, t, c in deferred.get((q, pair), {}).get(sk, ()):
                    emit_qk_proj(nm, t, c)
                if q == 0 and pair == 0:
                    emit_v_proj(sk)
                for fn in pending.pop((q, pair, sk), ()):
                    fn()
                pend_ctx.append((q, pair, sk, ex))
                if len(pend_ctx) >= 2:
                    emit_ctx(*pend_ctx.pop(0))

                if sk == ST - 1:
                    # Schedule this pair's epilogue inside the next pair,
                    # ONE op per step: a bursty normalize chain on the DVE
                    # would delay its exp completions, which gate the next
                    # scores via the PSUM rotation (PE stall + pstate drop).
                    nq_, np_ = (q, 1) if pair == 0 else (q + 1, 0)
                    in_steps = nq_ < NQ
                    state = {}

                    def t1(qq=q, pp=pair):
                        ctx_ps = ctx_tiles.pop((qq, pp))
                        state["ctx_ps"] = ctx_ps
                        state["ctxu"] = misc.tile(
                            [128, QW], F32, name=f"ctxu{qq}_{pp}", tag="ctxu")
                        state["den"] = misc.tile(
                            [2, QW], F32, name=f"den{qq}_{pp}", tag="den")
                        sched.evict(state["ctxu"][0:49, :], ctx_ps[0:49, :], QW)
                        nc.sync.dma_start(out=state["den"][0:1, :],
                                          in_=state["ctxu"][48:49, :])

                    def t2(qq=q, pp=pair):
                        sched.evict(state["ctxu"][64:113, :],
                                    state["ctx_ps"][64:113, :], QW)
                        nc.sync.dma_start(out=state["den"][1:2, :],
                                          in_=state["ctxu"][112:113, :])

                    def t3(qq=q, pp=pair):
                        state["recf"] = misc.tile(
                            [2, QW], F32, name=f"recf{qq}_{pp}", tag="recf")
                        state["rec"] = misc.tile(
                            [2, QW], BF16, name=f"rec{qq}_{pp}", tag="rec")
                        nc.vector.reciprocal_approx_fast(
                            out=state["recf"][:], in_=state["den"][:])
                        sched.dve_only(1.0417 * QW + 170)
                        sched.evict(state["rec"][:], state["recf"][:], QW)

                    def t4(qq=q, pp=pair):
                        bc = ps_aux.tile([128, QW], F32,
                                         name=f"bc{qq}_{pp}", tag="ps_aux")
                        state["bc"] = bc
                        nc.tensor.matmul(bc[:], lhsT=ones_sb[:],
                                         rhs=state["rec"][:],
                                         start=True, stop=True)
                        with nc.allow_low_precision(reason="bf16 ctxT"):
                            nc.vector.tensor_mul(
                                ctxT_sb[pp][0:48, qq * QW:(qq + 1) * QW],
                                state["ctxu"][0:48, :], bc[0:48, :])
                        sched.dve_only(1.0417 * QW + 170)

                    def t5(qq=q, pp=pair):
                        with nc.allow_low_precision(reason="bf16 ctxT"):
                            nc.vector.tensor_mul(
                                ctxT_sb[pp][64:112, qq * QW:(qq + 1) * QW],
                                state["ctxu"][64:112, :],
                                state["bc"][64:112, :])
                        sched.dve_only(1.0417 * QW + 170)

                    if in_steps:
                        for off, fn in ((1, t1), (2, t2), (5, t3),
                                        (6, t4), (7, t5)):
                            defer(nq_, np_, off, fn)
                    else:
                        while pend_ctx:            # drain the tail pipeline
                            emit_ctx(*pend_ctx.pop(0))
                        t1(); t2(); t3(); t4(); t5()
                    if pair == 1:
                        for sti in range(QW // 128):
                            st = q * (QW // 128) + sti
                            fn = (lambda s: lambda: out_proj_st(s))(st)
                            if q + 1 < NQ:
                                defer(q + 1, 0, 9 + 2 * sti, fn)
                            else:
                                fn()

    nc.compile()
    return nc


def _build_in_maps(x, Wq, bq, Wk, Wv, Wo):
    scale = 1.0 / np.sqrt(np.float32(DK))
    ones2 = np.zeros((2, 128), NPBF16)
    ones2[0, 0:64] = 1.0
    ones2[1, 64:128] = 1.0

    xT = [np.ascontiguousarray(x[b].T).astype(NPBF16) for b in range(B)]

    in_maps = []
    for core in range(NCORES):
        b, g = divmod(core, HPC)
        wq_pad = np.zeros((D, E), np.float32)
        wk_pad = np.zeros((D, E), np.float32)
        wv_pad = np.zeros((D, EV), np.float32)
        wo_pad = np.zeros((E, D), np.float32)
        for j in range(HPC):
            h = HPC * g + j
            sl = slice(DK * h, DK * (h + 1))
            wq_pad[:, 64 * j:64 * j + DK] = Wq[sl, :].T * scale
            wk_pad[:, 64 * j:64 * j + DK] = Wk[sl, :].T
            wk_pad[:, 64 * j + DK] = Wk[sl, :].T @ (bq[sl] * scale)
            wv_pad[:, 49 * j:49 * j + DK] = Wv[sl, :].T
            wo_pad[64 * j:64 * j + DK, :] = Wo[:, sl].T
        in_maps.append({
            "xT": xT[b],
            "wq": wq_pad.astype(NPBF16),
            "wk": wk_pad.astype(NPBF16),
            "wv": wv_pad.astype(NPBF16),
            "wo": wo_pad.astype(NPBF16),
            "ones2": ones2,
        })
    return in_maps


def _postprocess(results, Wo, bv, bo):
    const = (Wo @ bv + bo).astype(np.float32)  # folded V-bias + out bias
    out = np.empty((B, S, D), np.float32)
    for b in range(B):
        acc = results[HPC * b]["out"].astype(np.float32)
        for g in range(1, HPC):
            acc = acc + results[HPC * b + g]["out"].astype(np.float32)
        out[b] = acc + const
    return out


def get_program():
    global _PROGRAM
    if _PROGRAM is None:
        _PROGRAM = _build_program()
    return _PROGRAM


def kernel(x, Wq, bq, Wk, bk, Wv, bv, Wo, bo):
    x = np.asarray(x, np.float32)
    Wq, bq = np.asarray(Wq, np.float32), np.asarray(bq, np.float32)
    Wk = np.asarray(Wk, np.float32)
    Wv, bv = np.asarray(Wv, np.float32), np.asarray(bv, np.float32)
    Wo, bo = np.asarray(Wo, np.float32), np.asarray(bo, np.float32)

    nc = get_program()
    in_maps = _build_in_maps(x, Wq, bq, Wk, Wv, Wo)
    res = run_bass_kernel_spmd(nc, in_maps, list(range(NCORES)))
    return _postprocess(res.results, Wo, bv, bo)
